# revision 1
# baseline (speedup 1.0000x reference)
"""Trainium2 Bass kernel for nn_CNF_76355928588411.

Data-parallel over N across 8 NeuronCores. The tiny t-conditioned hypernet
(three dense layers -> W, U, gate, B; ~6.6 MFLOP, depends only on the scalar
t) is evaluated once on the host in fp32 and its ~49KB output is replicated
to all cores per the sharding hint. The N-compute (h = tanh(x@W^T + B)
[E=64, N], dx = h^T@U/E, plus the Jacobian-trace column; ~4.3 GFLOP and
~130MB of I/O) runs on the devices.

Per-core device pipeline, per window of 1024 samples (2 subs x 512 cols):
  mm1 x2 (f32r, K=128 zero-padded weight halves) -> psum hp [64, 1024]
  ACT tanh(+B per-partition bias)                -> H[0:64]    (fp16)
  DVE square (cross-partition-offset write)      -> H[64:128] = h^2 (fp16)
  mm2 x8: lhsT = H[:, 128c:128c+128] (fp16, K=128, M=128 samples),
          rhs = up' [128, 65] fp16               -> psum [128sample, 65ch]
  DVE tensor-tensor add (+const tile carrying -mean(wu) in channel 64)
                                                 -> out sbuf [128, 520] f32
  DMA out (2080B contiguous per partition via 8-way sample interleave)

up' rows 0:64 = [U/E | 0], rows 64:128 = [0 | wu/E], so one K=128 matmul
emits dx and the h^2-weighted Jacobian column together. All matmuls are
plain 128x128 mode (no tile_position) so the PE never switches tiling
modes. x stays f32r (fp16 x halves DMA but measured 1.38e-3 rel err vs
9.2e-4; kept the safer dtype). Host pre-permutes x so device loads are
plain slices and output stores are contiguous runs; x-batch DMAs are
issued ahead of the constant loads and a dummy tanh hoists the ACT table
load to t=0. TimelineSim: ~53.3us/core; DMA-bound (48us busy, ~96% stream utilization).
"""

import sys

sys.path.insert(0, "/opt/trn_rl_repo")

import numpy as np

import concourse.bass as bass
from concourse import bacc
import concourse.mybir as mybir
import concourse.tile as tile
from concourse.bass_utils import run_bass_kernel_spmd

F32 = mybir.dt.float32
F32R = mybir.dt.float32r
F16 = mybir.dt.float16
AF = mybir.ActivationFunctionType

E, D, H_DIM, N = 64, 64, 512, 262144
BLOCK = E * D
OUT_DIM = 3 * BLOCK + E
NCORES = 8
NSH = N // NCORES          # 32768 samples per core
WIN = 1024                 # samples per window
NWIN = NSH // WIN          # 32 windows
WQ = 2                     # windows per DMA batch
NQ = NWIN // WQ            # 8 DMA batches
CH = D + 1                 # 65 output channels

_CACHED = {}


def _build_nc():
    nc = bacc.Bacc("TRN2", target_bir_lowering=False, debug=False,
                   num_devices=NCORES)
    xt = nc.dram_tensor("xt", [128, NSH // 2], F32R, kind="ExternalInput")
    wtd = nc.dram_tensor("wtd", [128, 2 * D], F32R, kind="ExternalInput")
    up = nc.dram_tensor("up", [128, CH], F16, kind="ExternalInput")
    bvec = nc.dram_tensor("bvec", [E, 1], F32, kind="ExternalInput")
    cb = nc.dram_tensor("cb", [128, 8 * CH], F32, kind="ExternalInput")
    out = nc.dram_tensor("out", [NSH, CH], F32, kind="ExternalOutput")

    # out row = 1024*w + 8*p + c
    out_r = out.ap().rearrange("(w p c) ch -> w p c ch", p=128, c=8)

    with tile.TileContext(nc) as tc:
        with (
            tc.tile_pool(name="consts", bufs=1) as consts,
            tc.tile_pool(name="xin", bufs=8) as xin,
            tc.tile_pool(name="hh", bufs=4) as hhp,
            tc.tile_pool(name="outp", bufs=6) as outp,
            tc.tile_pool(name="ps_h", bufs=2, space="PSUM") as ps_h,
            tc.tile_pool(name="ps_o", bufs=2, space="PSUM") as ps_o,
        ):
            wtd_t = consts.tile([128, 2 * D], F32R)  # cols 0:64=[WT;0], 64:128=[0;WT]
            up_t = consts.tile([128, CH], F16)
            bvec_t = consts.tile([E, 1], F32)
            cb_t = consts.tile([128, 8 * CH], F32)
            xqs = {}

            def fetch(q, split=False):
                xq_t = xin.tile([128, WQ * 512], F32R, tag="xq")
                xqs[q] = xq_t
                lo = q * WQ * 512
                if split:
                    nc.sync.dma_start(out=xq_t[:, 0:512],
                                      in_=xt[:, lo:lo + 512])
                    nc.sync.dma_start(out=xq_t[:, 512:WQ * 512],
                                      in_=xt[:, lo + 512:lo + WQ * 512])
                else:
                    nc.sync.dma_start(out=xq_t, in_=xt[:, lo:lo + WQ * 512])

            fetch(0)
            dummy = consts.tile([1, 2], F32)
            nc.vector.memset(dummy, 0.0)
            nc.scalar.activation(dummy[:, 1:2], dummy[:, 0:1], AF.Tanh)
            nc.sync.dma_start(out=wtd_t, in_=wtd[:, :])
            nc.sync.dma_start(out=up_t, in_=up[:, :])
            nc.sync.dma_start(out=bvec_t, in_=bvec[:, :])
            nc.sync.dma_start(out=cb_t, in_=cb[:, :])
            fetch(1)
            fetch(2)
            fetch(3)
            fetch(4)

            for q in range(NQ):
                if q + 5 < NQ:
                    fetch(q + 5)
                xq = xqs.pop(q)
                for i in range(WQ):
                    ob = outp.tile([128, 8 * CH], F32)
                    xw = xq[:, i * 512:(i + 1) * 512]
                    hp = ps_h.tile([E, WIN], F32)
                    # K=128 with zero-padded weight halves -> plain mode
                    nc.tensor.matmul(hp[:, 0:512], wtd_t[:, 0:D], xw,
                                     start=True, stop=True)
                    nc.tensor.matmul(hp[:, 512:1024], wtd_t[:, D:2 * D], xw,
                                     start=True, stop=True)
                    hh = hhp.tile([128, WIN], F16)
                    nc.scalar.activation(hh[0:64, :], hp, AF.Tanh,
                                         bias=bvec_t[:, :], scale=1.0)
                    nc.vector.tensor_mul(hh[64:128, :], hh[0:64, :],
                                         hh[0:64, :])
                    po = ps_o.tile([128, WIN], F32)
                    for c in range(8):
                        nc.tensor.matmul(po[:, c * 128:c * 128 + CH],
                                         hh[:, c * 128:(c + 1) * 128],
                                         up_t[:, :], start=True, stop=True)
                    obw = ob.rearrange("p (c ch) -> p c ch", c=8)
                    po_v = po.rearrange("p (c j) -> p c j", c=8)[:, :, 0:CH]
                    cb_v = cb_t.rearrange("p (c ch) -> p c ch", c=8)
                    nc.vector.tensor_add(obw, po_v, cb_v)
                    nc.sync.dma_start(
                        out=out_r[q * WQ + i],
                        in_=ob.rearrange("p (c ch) -> p c ch", c=8),
                    )
    nc.compile()
    return nc


def _hypernet(t, W1, b1, W2, b2, W3, b3):
    p = np.tanh(t.reshape(1, 1) @ W1 + b1)
    p = np.tanh(p @ W2 + b2)
    p = (p @ W3 + b3).reshape(-1).astype(np.float32)
    W = p[:BLOCK].reshape(E, D)
    U = p[BLOCK:2 * BLOCK].reshape(E, D)
    G = 1.0 / (1.0 + np.exp(-p[2 * BLOCK:3 * BLOCK].reshape(E, D)))
    U = (U * G).astype(np.float32)
    B = p[3 * BLOCK:].reshape(E, 1).astype(np.float32)
    return W.astype(np.float32), U, B


def _host_layout_x(x):
    """[N, D] -> per-core device layouts [NCORES][128, NSH//2].

    Sample index within a 1024-window: 8*p + 4*s + a (p<128, s<2, a<4);
    stored at partition (s*64+d), column (w*512 + a*128 + p).
    """
    xs = x.reshape(NCORES, NWIN, 128, 2, 4, D)        # [core, w, p, s, a, d]
    xs = xs.transpose(0, 3, 5, 1, 4, 2)               # [core, s, d, w, a, p]
    return np.ascontiguousarray(xs).reshape(NCORES, 128, NSH // 2)


def kernel(t, x, W1, b1, W2, b2, W3, b3):
    W, U, B = _hypernet(
        np.asarray(t, np.float32), np.asarray(W1, np.float32),
        np.asarray(b1, np.float32), np.asarray(W2, np.float32),
        np.asarray(b2, np.float32), np.asarray(W3, np.float32),
        np.asarray(b3, np.float32),
    )
    wu = np.sum(W * U, axis=1).astype(np.float32)      # [E]

    wtd = np.zeros((128, 2 * D), np.float32)
    wtd[0:64, 0:D] = W.T
    wtd[64:128, D:2 * D] = W.T
    up = np.zeros((128, CH), np.float32)
    up[0:E, 0:D] = U / E
    up[E:128, D] = wu / E
    up = up.astype(np.float16)
    cb = np.zeros((128, 8 * CH), np.float32)
    cb[:, D::CH] = -np.sum(wu) / E
    bvec = B.reshape(E, 1).astype(np.float32)

    xl = _host_layout_x(np.asarray(x, np.float32))

    if "nc" not in _CACHED:
        _CACHED["nc"] = _build_nc()
    nc = _CACHED["nc"]

    in_maps = [
        {"xt": xl[c], "wtd": wtd, "up": up, "bvec": bvec, "cb": cb}
        for c in range(NCORES)
    ]
    res = run_bass_kernel_spmd(nc, in_maps, core_ids=list(range(NCORES)))
    outs = [res.results[c]["out"] for c in range(NCORES)]
    return np.concatenate(outs, axis=0)



# revision 13
# speedup vs baseline: 1.1614x; 1.1614x over previous
"""Trainium2 Bass kernel for nn_CNF_76355928588411.

Data-parallel over N across 8 NeuronCores. The tiny t-conditioned hypernet
(three dense layers -> W, U, gate, B; depends only on the scalar t) is
evaluated once on the host in fp32; its ~50KB of derived weights are
replicated to all cores. The N-compute (h = tanh(x@W^T + B), dx = h^T@U/E,
Jacobian-trace column) runs on the devices.

Layout: windows of 1024 samples packed as [128, 512] tiles - two sample
groups (s=0,1) stacked on the partition dim, so every matmul uses the full
128x128 PE array via block-diagonal weights:
  mm1: hp = blockdiag(W^T, W^T) @ xw          [128, 512] psum (512 fp16 rows)
  ACT: t1 = tanh(hp + [B;B])                  [128, 512] fp16
  DVE: t2 = t1*t1 (2x fp16 mode, per pair)    [128, 1024] fp16
  mm2: po = blockdiag(U/E, U/E)^T @ t1 -> dx  [128ch, 512] psum
  mm3: dl = [wu|0 ; 0|wu]^T @ t2              [2, 512] psum (raw sum wu*h^2)
  ACT+DVE: po f32 -> ob fp16 (217/295 col split, one instr per 2 windows)
  DMA: ob -> dxh fp16; dl -> dlh f32 (host applies (dl - sum wu)/E)

All I/O is fp16 except the tiny dl column (f32). GPSIMD cannot touch PSUM
and DMA cannot read PSUM, so the dx egress (512 cols/window) must share
ACT+DVE with tanh/square - that egress is the ~885ns/window critical
resource; DMA (~853ns/window incl. dl garbage rows) and PE (~640) sit just
under it. dl matmuls for groups of 3 windows write one [66, 512] psum tile
at partition bases {0,32,64}; one DVE copy stages the group to SBUF and a
single [66, 512] DMA (4 dead row-pairs) emits it. po/t1/t2 tiles span 2
windows so the psum access bubbles amortize; PSUM = 2+2*2+2 = 8 banks.
"""

import sys

sys.path.insert(0, "/opt/trn_rl_repo")

import numpy as np

import concourse.bass as bass
from concourse import bacc
import concourse.mybir as mybir
import concourse.tile as tile
from concourse.bass_utils import run_bass_kernel_spmd

F32 = mybir.dt.float32
F16 = mybir.dt.float16
AF = mybir.ActivationFunctionType

E, D, H_DIM, N = 64, 64, 512, 262144
BLOCK = E * D
NCORES = 8
NSH = N // NCORES          # 32768 samples per core
WIN = 1024                 # samples per window ([128, 512] dual-packed)
NWIN = NSH // WIN          # 32 windows
WQ = 4                     # windows per x/dx DMA batch
NQ = NWIN // WQ            # 8 DMA batches
CA = 224                   # dx egress cols per window copied by ACT
DLG = 3                    # windows per dl psum group tile
NDLG = (NWIN + DLG - 1) // DLG   # 11 dl groups

_CACHED = {}


def _build_nc():
    nc = bacc.Bacc("TRN2", target_bir_lowering=False, debug=False,
                   num_devices=NCORES)
    xt = nc.dram_tensor("xt", [128, NSH // 2], F16, kind="ExternalInput")
    # cst cols: 0:128 Wblk, 128:256 UPblk, 256:258 wublk
    cst = nc.dram_tensor("cst", [128, 258], F16, kind="ExternalInput")
    bdup = nc.dram_tensor("bdup", [128, 1], F32, kind="ExternalInput")
    dxh = nc.dram_tensor("dxh", [128, NSH // 2], F16, kind="ExternalOutput")
    dlh = nc.dram_tensor("dlh", [66, 512 * NDLG], F32, kind="ExternalOutput")

    with tile.TileContext(nc) as tc:
        with (
            tc.tile_pool(name="consts", bufs=1) as consts,
            tc.tile_pool(name="xin", bufs=4) as xin,
            tc.tile_pool(name="t1p", bufs=2) as t1p,
            tc.tile_pool(name="t2p", bufs=2) as t2p,
            tc.tile_pool(name="dlsp", bufs=2) as dlsp,
            tc.tile_pool(name="obp", bufs=2) as obp,
            tc.tile_pool(name="ps_h", bufs=2, space="PSUM") as ps_h,
            tc.tile_pool(name="ps_o", bufs=2, space="PSUM") as ps_o,
            tc.tile_pool(name="ps_dl", bufs=2, space="PSUM") as ps_dl,
        ):
            cst_t = consts.tile([128, 258], F16)
            bdup_t = consts.tile([128, 1], F32)
            xqs = {}

            def fetch(q):
                xq_t = xin.tile([128, WQ * 512], F16, tag="xq")
                xqs[q] = xq_t
                lo = q * WQ * 512
                nc.sync.dma_start(out=xq_t, in_=xt[:, lo:lo + WQ * 512])

            # warm the ACT table at t=0
            dummy = consts.tile([1, 2], F32)
            nc.vector.memset(dummy, 0.0)
            nc.scalar.activation(dummy[:, 1:2], dummy[:, 0:1], AF.Tanh)

            nc.sync.dma_start(out=cst_t, in_=cst[:, :])
            nc.sync.dma_start(out=bdup_t, in_=bdup[:, :])
            fetch(0)
            fetch(1)
            fetch(2)

            wblk = cst_t[:, 0:128]
            upblk = cst_t[:, 128:256]
            wublk = cst_t[:, 256:258]

            t1_cur = {}    # current 2-window t1 tile
            po_cur = {}    # current 2-window po tile
            dl_cur = {}    # current dl group psum tile
            ob_cur = {}    # current ob batch tile

            for it in range(NWIN + 2):
                # stage A: window it -> mm1, tanh; square per pair
                if it < NWIN:
                    w = it
                    if w % WQ == 0 and w // WQ + 3 < NQ:
                        fetch(w // WQ + 3)
                    xq = xqs[w // WQ]
                    xw = xq[:, (w % WQ) * 512:(w % WQ + 1) * 512]
                    hp = ps_h.tile([128, 512], F32)
                    nc.tensor.matmul(hp, wblk, xw, start=True, stop=True)
                    if w % 2 == 0:
                        t1_cur[w // 2] = t1p.tile([128, 1024], F16, name="t1")
                    t1 = t1_cur[w // 2]
                    half = (w % 2) * 512
                    nc.scalar.activation(t1[:, half:half + 512], hp, AF.Tanh,
                                         bias=bdup_t[:, :], scale=1.0)
                    if w % 2 == 1:
                        t2 = t2p.tile([128, 1024], F16, name="t2")
                        nc.vector.tensor_mul(t2, t1, t1)
                        t1_cur[w // 2] = (t1, t2)
                    if w % WQ == 3:
                        del xqs[w // WQ]
                # stage B: window it-2 -> mm2, dl matmul
                v = it - 2
                if v < 0:
                    continue
                t1, t2 = t1_cur[v // 2]
                if v % 2 == 0:
                    po_cur[0] = ps_o.tile([128, 1024], F32, name="po")
                po = po_cur[0]
                half = (v % 2) * 512
                nc.tensor.matmul(po[:, half:half + 512], upblk,
                                 t1[:, half:half + 512], start=True, stop=True)
                g, k = v // DLG, v % DLG
                if k == 0:
                    dl_cur[0] = ps_dl.tile([66, 512], F32, name="dl")
                dl = dl_cur[0]
                nc.tensor.matmul(dl[32 * k:32 * k + 2, :], wublk,
                                 t2[:, half:half + 512], start=True, stop=True)
                if v % 2 == 1:
                    del t1_cur[v // 2]
                    # dx egress for the completed pair (windows v-1, v)
                    if (v - 1) % WQ == 0:
                        ob_cur[0] = obp.tile([128, WQ * 512], F16, name="ob")
                    ob = ob_cur[0]
                    base = ((v - 1) % WQ) * 512
                    po_v = po.rearrange("p (h c) -> p h c", h=2)
                    ob_v = ob.rearrange("p (q c) -> p q c", q=WQ)[
                        :, (v - 1) % WQ:(v - 1) % WQ + 2]
                    nc.scalar.copy(ob_v[:, :, 0:CA], po_v[:, :, 0:CA])
                    nc.vector.tensor_copy(ob_v[:, :, CA:512],
                                          po_v[:, :, CA:512])
                if k == DLG - 1 or v == NWIN - 1:
                    dls = dlsp.tile([66, 512], F32, name="dls", tag="dls")
                    nc.vector.tensor_copy(dls, dl)
                    nc.scalar.dma_start(
                        out=dlh[:, g * 512:(g + 1) * 512], in_=dls)
                if v % WQ == WQ - 1:
                    lo = (v // WQ) * WQ * 512
                    nc.sync.dma_start(out=dxh[:, lo:lo + WQ * 512], in_=ob)
    nc.compile()
    return nc


def _hypernet(t, W1, b1, W2, b2, W3, b3):
    p = np.tanh(t.reshape(1, 1) @ W1 + b1)
    p = np.tanh(p @ W2 + b2)
    p = (p @ W3 + b3).reshape(-1).astype(np.float32)
    W = p[:BLOCK].reshape(E, D)
    U = p[BLOCK:2 * BLOCK].reshape(E, D)
    G = 1.0 / (1.0 + np.exp(-p[2 * BLOCK:3 * BLOCK].reshape(E, D)))
    U = (U * G).astype(np.float32)
    B = p[3 * BLOCK:].reshape(E, 1).astype(np.float32)
    return W.astype(np.float32), U, B


def kernel(t, x, W1, b1, W2, b2, W3, b3):
    W, U, B = _hypernet(
        np.asarray(t, np.float32), np.asarray(W1, np.float32),
        np.asarray(b1, np.float32), np.asarray(W2, np.float32),
        np.asarray(b2, np.float32), np.asarray(W3, np.float32),
        np.asarray(b3, np.float32),
    )
    wu = np.sum(W * U, axis=1).astype(np.float32)      # [E]

    cst = np.zeros((128, 258), np.float32)
    cst[0:64, 0:64] = W.T
    cst[64:128, 64:128] = W.T
    cst[0:64, 128:192] = U / E
    cst[64:128, 192:256] = U / E
    cst[0:64, 256] = wu
    cst[64:128, 257] = wu
    cst = cst.astype(np.float16)
    bdup = np.concatenate([B, B], axis=0).reshape(128, 1).astype(np.float32)

    # x [N, D] -> per-core [128, NSH//2] fp16; sample (c, w, s, j) at
    # partition s*64+d, column w*512+j
    xs = np.asarray(x, np.float16).reshape(NCORES, NWIN, 2, 512, D)
    xs = np.ascontiguousarray(xs.transpose(0, 2, 4, 1, 3))
    xl = xs.reshape(NCORES, 128, NSH // 2)

    if "nc" not in _CACHED:
        _CACHED["nc"] = _build_nc()
    nc = _CACHED["nc"]

    in_maps = [
        {"xt": xl[c], "cst": cst, "bdup": bdup}
        for c in range(NCORES)
    ]
    res = run_bass_kernel_spmd(nc, in_maps, core_ids=list(range(NCORES)))

    out = np.empty((N, D + 1), np.float32)
    od = out[:, :D].reshape(NCORES, NWIN, 2, 512, D)
    ol = out[:, D].reshape(NCORES, NWIN, 2, 512)
    sw = float(np.sum(wu))
    for c in range(NCORES):
        dxc = res.results[c]["dxh"].astype(np.float32)
        od[c] = dxc.reshape(2, D, NWIN, 512).transpose(2, 0, 3, 1)
        dlc = res.results[c]["dlh"]          # [66, 512*NDLG]
        # window w = g*DLG + k lives at rows 32k:32k+2, group col block g
        dlg = dlc.reshape(33, 2, NDLG, 512)[::16]      # [k, s, g, j]
        dlw = dlg.transpose(2, 0, 1, 3).reshape(NDLG * DLG, 2, 512)[:NWIN]
        ol[c] = (dlw - sw) / E
    return out


# revision 14
# speedup vs baseline: 1.1677x; 1.0054x over previous
"""Trainium2 Bass kernel for nn_CNF_76355928588411.

Data-parallel over N across 8 NeuronCores. The tiny t-conditioned hypernet
(three dense layers -> W, U, gate, B; depends only on the scalar t) is
evaluated once on the host in fp32; its ~50KB of derived weights are
replicated to all cores. The N-compute (h = tanh(x@W^T + B), dx = h^T@U/E,
Jacobian-trace column) runs on the devices.

Layout: windows of 1024 samples packed as [128, 512] tiles - two sample
groups (s=0,1) stacked on the partition dim, so every matmul uses the full
128x128 PE array via block-diagonal weights:
  mm1: hp = blockdiag(W^T, W^T) @ xw          [128, 512] psum (512 fp16 rows)
  ACT: t1 = tanh(hp + [B;B])                  [128, 512] fp16
  DVE: t2 = t1*t1 (2x fp16 mode, per pair)    [128, 1024] fp16
  mm2: po = blockdiag(U/E, U/E)^T @ t1 -> dx  [128ch, 512] psum
  mm3: dl = [wu|0 ; 0|wu]^T @ t2              [2, 512] psum (raw sum wu*h^2)
  ACT+DVE: po f32 -> ob fp16 (224/288 col split, one instr per 2 windows)
  DMA: ob -> dxh fp16; dl -> dlh f32 (host applies (dl - sum wu)/E)

All I/O is fp16 except the tiny dl column (f32). GPSIMD cannot touch PSUM
and DMA cannot read PSUM, so the dx egress (512 cols/window) must share
ACT+DVE with tanh/square - that egress is the ~885ns/window critical
resource; DMA (~853ns/window incl. dl garbage rows) and PE (~640) sit just
under it. dl matmuls for groups of 3 windows write one [66, 512] psum tile
at partition bases {0,32,64}; one DVE copy stages the group to SBUF and a
single [66, 512] DMA (4 dead row-pairs) emits it. po/t1/t2 tiles span 2
windows so the psum access bubbles amortize; PSUM = 2+2*2+2 = 8 banks.
The mm2/dl/egress stage trails mm1/tanh by 3 windows so the PE's in-order
queue never stalls on the tanh->square chain; the first x fetch is split so
window 0 starts ~2.5us earlier; the last dx batch DMAs per-pair to shrink
the drain.
"""

import sys

sys.path.insert(0, "/opt/trn_rl_repo")

import numpy as np

import concourse.bass as bass
from concourse import bacc
import concourse.mybir as mybir
import concourse.tile as tile
from concourse.bass_utils import run_bass_kernel_spmd

F32 = mybir.dt.float32
F16 = mybir.dt.float16
AF = mybir.ActivationFunctionType

E, D, H_DIM, N = 64, 64, 512, 262144
BLOCK = E * D
NCORES = 8
NSH = N // NCORES          # 32768 samples per core
WIN = 1024                 # samples per window ([128, 512] dual-packed)
NWIN = NSH // WIN          # 32 windows
WQ = 4                     # windows per x/dx DMA batch
NQ = NWIN // WQ            # 8 DMA batches
CA = 224                   # dx egress cols per window copied by ACT
DLG = 3                    # windows per dl psum group tile
NDLG = (NWIN + DLG - 1) // DLG   # 11 dl groups
SKEW = 3                   # iterations between mm1 and mm2 stages

_CACHED = {}


def _build_nc():
    nc = bacc.Bacc("TRN2", target_bir_lowering=False, debug=False,
                   num_devices=NCORES)
    xt = nc.dram_tensor("xt", [128, NSH // 2], F16, kind="ExternalInput")
    # cst cols: 0:128 Wblk, 128:256 UPblk, 256:258 wublk, 258:260 B (f32 bits)
    cst = nc.dram_tensor("cst", [128, 260], F16, kind="ExternalInput")
    dxh = nc.dram_tensor("dxh", [128, NSH // 2], F16, kind="ExternalOutput")
    dlh = nc.dram_tensor("dlh", [66, 512 * NDLG], F32, kind="ExternalOutput")

    with tile.TileContext(nc) as tc:
        with (
            tc.tile_pool(name="consts", bufs=1) as consts,
            tc.tile_pool(name="xin", bufs=4) as xin,
            tc.tile_pool(name="t1p", bufs=3) as t1p,
            tc.tile_pool(name="t2p", bufs=3) as t2p,
            tc.tile_pool(name="dlsp", bufs=2) as dlsp,
            tc.tile_pool(name="obp", bufs=2) as obp,
            tc.tile_pool(name="ps_h", bufs=2, space="PSUM") as ps_h,
            tc.tile_pool(name="ps_o", bufs=2, space="PSUM") as ps_o,
            tc.tile_pool(name="ps_dl", bufs=2, space="PSUM") as ps_dl,
        ):
            cst_t = consts.tile([128, 260], F16)
            xqs = {}

            def fetch(q, split=False):
                xq_t = xin.tile([128, WQ * 512], F16, tag="xq")
                xqs[q] = xq_t
                lo = q * WQ * 512
                if split:
                    nc.sync.dma_start(out=xq_t[:, 0:512],
                                      in_=xt[:, lo:lo + 512])
                    nc.sync.dma_start(out=xq_t[:, 512:WQ * 512],
                                      in_=xt[:, lo + 512:lo + WQ * 512])
                else:
                    nc.sync.dma_start(out=xq_t, in_=xt[:, lo:lo + WQ * 512])

            # warm the ACT table at t=0 (hoists the 1.3us table load)
            dummy = consts.tile([1, 2], F32)
            nc.vector.memset(dummy, 0.0)
            nc.scalar.activation(dummy[:, 1:2], dummy[:, 0:1], AF.Tanh)

            nc.sync.dma_start(out=cst_t, in_=cst[:, :])
            fetch(0, split=True)
            fetch(1)
            fetch(2)

            wblk = cst_t[:, 0:128]
            upblk = cst_t[:, 128:256]
            wublk = cst_t[:, 256:258]
            bdup_t = cst_t[:, 258:260].bitcast(F32)

            t1_cur = {}    # pair index -> t1 tile / (t1, t2)
            po_cur = {}    # current 2-window po tile
            dl_cur = {}    # current dl group psum tile
            ob_cur = {}    # current ob batch tile

            for it in range(NWIN + SKEW):
                # stage A: window it -> mm1, tanh; square per pair
                if it < NWIN:
                    w = it
                    if w % WQ == 1 and w // WQ + 3 < NQ:
                        fetch(w // WQ + 3)
                    xq = xqs[w // WQ]
                    xw = xq[:, (w % WQ) * 512:(w % WQ + 1) * 512]
                    hp = ps_h.tile([128, 512], F32)
                    nc.tensor.matmul(hp, wblk, xw, start=True, stop=True)
                    if w % 2 == 0:
                        t1_cur[w // 2] = t1p.tile([128, 1024], F16, name="t1")
                    t1 = t1_cur[w // 2]
                    half = (w % 2) * 512
                    nc.scalar.activation(t1[:, half:half + 512], hp, AF.Tanh,
                                         bias=bdup_t, scale=1.0)
                    if w % 2 == 1:
                        t2 = t2p.tile([128, 1024], F16, name="t2")
                        nc.vector.tensor_mul(t2, t1, t1)
                        t1_cur[w // 2] = (t1, t2)
                    if w % WQ == 3:
                        del xqs[w // WQ]
                # stage B: window it-SKEW -> mm2, dl matmul, egress
                v = it - SKEW
                if v < 0:
                    continue
                t1, t2 = t1_cur[v // 2]
                if v % 2 == 0:
                    po_cur[0] = ps_o.tile([128, 1024], F32, name="po")
                po = po_cur[0]
                half = (v % 2) * 512
                nc.tensor.matmul(po[:, half:half + 512], upblk,
                                 t1[:, half:half + 512], start=True, stop=True)
                g, k = v // DLG, v % DLG
                if k == 0:
                    dl_cur[0] = ps_dl.tile([66, 512], F32, name="dl")
                dl = dl_cur[0]
                nc.tensor.matmul(dl[32 * k:32 * k + 2, :], wublk,
                                 t2[:, half:half + 512], start=True, stop=True)
                if v % 2 == 1:
                    del t1_cur[v // 2]
                    # dx egress for the completed pair (windows v-1, v)
                    if (v - 1) % WQ == 0:
                        ob_cur[0] = obp.tile([128, WQ * 512], F16, name="ob")
                    ob = ob_cur[0]
                    pr = ((v - 1) % WQ) // 2
                    po_v = po.rearrange("p (h c) -> p h c", h=2)
                    ob_v = ob.rearrange("p (q c) -> p q c", q=WQ)[
                        :, 2 * pr:2 * pr + 2]
                    nc.scalar.copy(ob_v[:, :, 0:CA], po_v[:, :, 0:CA])
                    nc.vector.tensor_copy(ob_v[:, :, CA:512],
                                          po_v[:, :, CA:512])
                if k == DLG - 1 or v == NWIN - 1:
                    dls = dlsp.tile([66, 512], F32, name="dls", tag="dls")
                    nc.vector.tensor_copy(dls, dl)
                    nc.scalar.dma_start(
                        out=dlh[:, g * 512:(g + 1) * 512], in_=dls)
                last_batch = v // WQ == NQ - 1
                if v % WQ == WQ - 1 and not last_batch:
                    lo = (v // WQ) * WQ * 512
                    nc.sync.dma_start(out=dxh[:, lo:lo + WQ * 512], in_=ob)
                elif last_batch and v % 2 == 1:
                    # final batch: emit per pair to shrink the drain
                    lo = (v - 1) * 512
                    pr = ((v - 1) % WQ) // 2
                    nc.sync.dma_start(
                        out=dxh[:, lo:lo + 1024],
                        in_=ob[:, pr * 1024:(pr + 1) * 1024])
    nc.compile()
    return nc


def _hypernet(t, W1, b1, W2, b2, W3, b3):
    p = np.tanh(t.reshape(1, 1) @ W1 + b1)
    p = np.tanh(p @ W2 + b2)
    p = (p @ W3 + b3).reshape(-1).astype(np.float32)
    W = p[:BLOCK].reshape(E, D)
    U = p[BLOCK:2 * BLOCK].reshape(E, D)
    G = 1.0 / (1.0 + np.exp(-p[2 * BLOCK:3 * BLOCK].reshape(E, D)))
    U = (U * G).astype(np.float32)
    B = p[3 * BLOCK:].reshape(E, 1).astype(np.float32)
    return W.astype(np.float32), U, B


def kernel(t, x, W1, b1, W2, b2, W3, b3):
    W, U, B = _hypernet(
        np.asarray(t, np.float32), np.asarray(W1, np.float32),
        np.asarray(b1, np.float32), np.asarray(W2, np.float32),
        np.asarray(b2, np.float32), np.asarray(W3, np.float32),
        np.asarray(b3, np.float32),
    )
    wu = np.sum(W * U, axis=1).astype(np.float32)      # [E]

    cst = np.zeros((128, 258), np.float32)
    cst[0:64, 0:64] = W.T
    cst[64:128, 64:128] = W.T
    cst[0:64, 128:192] = U / E
    cst[64:128, 192:256] = U / E
    cst[0:64, 256] = wu
    cst[64:128, 257] = wu
    cst = cst.astype(np.float16)
    bdup = np.concatenate([B, B], axis=0).reshape(128, 1).astype(np.float32)
    cst = np.concatenate([cst, bdup.view(np.float16).reshape(128, 2)], axis=1)

    # x [N, D] -> per-core [128, NSH//2] fp16; sample (c, w, s, j) at
    # partition s*64+d, column w*512+j
    xs = np.asarray(x, np.float16).reshape(NCORES, NWIN, 2, 512, D)
    xs = np.ascontiguousarray(xs.transpose(0, 2, 4, 1, 3))
    xl = xs.reshape(NCORES, 128, NSH // 2)

    if "nc" not in _CACHED:
        _CACHED["nc"] = _build_nc()
    nc = _CACHED["nc"]

    in_maps = [
        {"xt": xl[c], "cst": cst}
        for c in range(NCORES)
    ]
    res = run_bass_kernel_spmd(nc, in_maps, core_ids=list(range(NCORES)))

    out = np.empty((N, D + 1), np.float32)
    od = out[:, :D].reshape(NCORES, NWIN, 2, 512, D)
    ol = out[:, D].reshape(NCORES, NWIN, 2, 512)
    sw = float(np.sum(wu))
    for c in range(NCORES):
        dxc = res.results[c]["dxh"].astype(np.float32)
        od[c] = dxc.reshape(2, D, NWIN, 512).transpose(2, 0, 3, 1)
        dlc = res.results[c]["dlh"]          # [66, 512*NDLG]
        # window w = g*DLG + k lives at rows 32k:32k+2, group col block g
        dlg = dlc.reshape(33, 2, NDLG, 512)[::16]      # [k, s, g, j]
        dlw = dlg.transpose(2, 0, 1, 3).reshape(NDLG * DLG, 2, 512)[:NWIN]
        ol[c] = (dlw - sw) / E
    return out


# revision 15
# speedup vs baseline: 1.3030x; 1.1158x over previous
"""Trainium2 Bass kernel for nn_CNF_76355928588411.

Data-parallel over N across 8 NeuronCores. The tiny t-conditioned hypernet
(three dense layers -> W, U, gate, B; depends only on the scalar t) is
evaluated once on the host in fp32; its ~50KB of derived weights are
replicated to all cores. The N-compute (h = tanh(x@W^T + B), dx = h^T@U/E,
Jacobian-trace column) runs on the devices.

Layout: windows of 1024 samples packed as [128, 512] tiles - two sample
groups (s=0,1) stacked on the partition dim, so every matmul uses the full
128x128 PE array via block-diagonal weights:
  mm1: hp = blockdiag(W^T, W^T) @ xw          [128, 512] psum (512 fp16 rows)
  ACT: t1 = tanh(hp + [B;B])                  [128, 512] fp16
  DVE: t2 = t1*t1 (2x fp16 mode, per pair)    [128, 1024] fp16
  mm2: po = blockdiag(U/E, U/E)^T @ t1 -> dx  [128ch, 512] psum
  mm3: dl = [wu|0 ; 0|wu]^T @ t2              [2, 512] psum (raw sum wu*h^2)
  ACT+DVE: po f32 -> ob fp16 (224/288 col split, one instr per 2 windows)
  DMA: ob -> dxh fp16; dl -> dlh f32 (host applies (dl - sum wu)/E)

All I/O is fp16 except the tiny dl column (f32). GPSIMD cannot touch PSUM
and DMA cannot read PSUM, so the dx egress (512 cols/window) must share
ACT+DVE with tanh/square - that egress is the ~885ns/window critical
resource; DMA (~853ns/window incl. dl garbage rows) and PE (~640) sit just
under it. dl matmuls for groups of 3 windows write one [66, 512] psum tile
at partition bases {0,32,64}; one DVE copy stages the group to SBUF and a
single [66, 512] DMA (4 dead row-pairs) emits it. po/t1/t2 tiles span 2
windows so the psum access bubbles amortize; PSUM = 2+2*2+2 = 8 banks.
The mm2/dl/egress stage trails mm1/tanh by 3 windows so the PE's in-order
queue never stalls on the tanh->square chain; the first x fetch is split so
window 0 starts ~2.5us earlier; the last dx batch DMAs per-pair to shrink
the drain.
"""

import sys

sys.path.insert(0, "/opt/trn_rl_repo")

import numpy as np

import concourse.bass as bass
from concourse import bacc
import concourse.mybir as mybir
import concourse.tile as tile
from concourse.bass_utils import run_bass_kernel_spmd

F32 = mybir.dt.float32
F16 = mybir.dt.float16
AF = mybir.ActivationFunctionType

E, D, H_DIM, N = 64, 64, 512, 262144
BLOCK = E * D
NCORES = 8
NSH = N // NCORES          # 32768 samples per core
WIN = 1024                 # samples per window ([128, 512] dual-packed)
NWIN = NSH // WIN          # 32 windows
WQ = 4                     # windows per x/dx DMA batch
NQ = NWIN // WQ            # 8 DMA batches
CA = 224                   # dx egress cols per window copied by ACT
DLG = 3                    # windows per dl psum group tile
NDLG = (NWIN + DLG - 1) // DLG   # 11 dl groups
SKEW = 3                   # iterations between mm1 and mm2 stages

_CACHED = {}


def _build_nc():
    nc = bacc.Bacc("TRN2", target_bir_lowering=False, debug=False,
                   num_devices=NCORES)
    xt = nc.dram_tensor("xt", [128, NSH // 2], F16, kind="ExternalInput")
    # cst cols: 0:128 Wblk, 128:256 UPblk, 256:258 wublk, 258:260 B (f32 bits)
    cst = nc.dram_tensor("cst", [128, 260], F16, kind="ExternalInput")
    dxh = nc.dram_tensor("dxh", [128, NSH // 2], F16, kind="ExternalOutput")
    dlh = nc.dram_tensor("dlh", [66, 512 * NDLG], F32, kind="ExternalOutput")

    with tile.TileContext(nc) as tc:
        with (
            tc.tile_pool(name="consts", bufs=1) as consts,
            tc.tile_pool(name="xin", bufs=4) as xin,
            tc.tile_pool(name="t1p", bufs=3) as t1p,
            tc.tile_pool(name="t2p", bufs=3) as t2p,
            tc.tile_pool(name="dlsp", bufs=2) as dlsp,
            tc.tile_pool(name="obp", bufs=2) as obp,
            tc.tile_pool(name="ps_h", bufs=2, space="PSUM") as ps_h,
            tc.tile_pool(name="ps_o", bufs=2, space="PSUM") as ps_o,
            tc.tile_pool(name="ps_dl", bufs=2, space="PSUM") as ps_dl,
        ):
            cst_t = consts.tile([128, 260], F16)
            xqs = {}

            def fetch(q, split=False):
                xq_t = xin.tile([128, WQ * 512], F16, tag="xq")
                xqs[q] = xq_t
                lo = q * WQ * 512
                if split:
                    nc.sync.dma_start(out=xq_t[:, 0:512],
                                      in_=xt[:, lo:lo + 512])
                    nc.sync.dma_start(out=xq_t[:, 512:WQ * 512],
                                      in_=xt[:, lo + 512:lo + WQ * 512])
                else:
                    nc.sync.dma_start(out=xq_t, in_=xt[:, lo:lo + WQ * 512])

            # warm the ACT table at t=0 (hoists the 1.3us table load)
            dummy = consts.tile([1, 2], F32)
            nc.vector.memset(dummy, 0.0)
            nc.scalar.activation(dummy[:, 1:2], dummy[:, 0:1], AF.Tanh)

            nc.sync.dma_start(out=cst_t, in_=cst[:, :])
            fetch(0, split=True)
            fetch(1)
            fetch(2)

            wblk = cst_t[:, 0:128]
            upblk = cst_t[:, 128:256]
            wublk = cst_t[:, 256:258]
            bdup_t = cst_t[:, 258:260].bitcast(F32)

            t1_cur = {}    # pair index -> t1 tile / (t1, t2)
            po_cur = {}    # current 2-window po tile
            dl_cur = {}    # current dl group psum tile
            ob_cur = {}    # current ob batch tile

            for it in range(NWIN + SKEW):
                # stage A: window it -> mm1, tanh; square per pair
                if it < NWIN:
                    w = it
                    if w % WQ == 1 and w // WQ + 3 < NQ:
                        fetch(w // WQ + 3)
                    xq = xqs[w // WQ]
                    xw = xq[:, (w % WQ) * 512:(w % WQ + 1) * 512]
                    hp = ps_h.tile([128, 512], F32)
                    nc.tensor.matmul(hp, wblk, xw, start=True, stop=True)
                    if w % 2 == 0:
                        t1_cur[w // 2] = t1p.tile([128, 1024], F16, name="t1")
                    t1 = t1_cur[w // 2]
                    half = (w % 2) * 512
                    nc.scalar.activation(t1[:, half:half + 512], hp, AF.Tanh,
                                         bias=bdup_t, scale=1.0)
                    if w % 2 == 1:
                        t2 = t2p.tile([128, 1024], F16, name="t2")
                        nc.vector.tensor_mul(t2, t1, t1)
                        t1_cur[w // 2] = (t1, t2)
                    if w % WQ == 3:
                        del xqs[w // WQ]
                # stage B: window it-SKEW -> mm2, dl matmul, egress
                v = it - SKEW
                if v < 0:
                    continue
                t1, t2 = t1_cur[v // 2]
                if v % 2 == 0:
                    po_cur[0] = ps_o.tile([128, 1024], F32, name="po")
                po = po_cur[0]
                half = (v % 2) * 512
                nc.tensor.matmul(po[:, half:half + 512], upblk,
                                 t1[:, half:half + 512], start=True, stop=True)
                g, k = v // DLG, v % DLG
                if k == 0:
                    dl_cur[0] = ps_dl.tile([66, 512], F32, name="dl")
                dl = dl_cur[0]
                nc.tensor.matmul(dl[32 * k:32 * k + 2, :], wublk,
                                 t2[:, half:half + 512], start=True, stop=True)
                if v % 2 == 1:
                    del t1_cur[v // 2]
                    # dx egress for the completed pair (windows v-1, v)
                    if (v - 1) % WQ == 0:
                        ob_cur[0] = obp.tile([128, WQ * 512], F16, name="ob")
                    ob = ob_cur[0]
                    pr = ((v - 1) % WQ) // 2
                    po_v = po.rearrange("p (h c) -> p h c", h=2)
                    ob_v = ob.rearrange("p (q c) -> p q c", q=WQ)[
                        :, 2 * pr:2 * pr + 2]
                    nc.scalar.copy(ob_v[:, :, 0:CA], po_v[:, :, 0:CA])
                    nc.vector.tensor_copy(ob_v[:, :, CA:512],
                                          po_v[:, :, CA:512])
                if k == DLG - 1 or v == NWIN - 1:
                    dls = dlsp.tile([66, 512], F32, name="dls", tag="dls")
                    nc.vector.tensor_copy(dls, dl)
                    nc.gpsimd.dma_start(
                        out=dlh[:, g * 512:(g + 1) * 512], in_=dls)
                last_batch = v // WQ == NQ - 1
                if v % WQ == WQ - 1 and not last_batch:
                    lo = (v // WQ) * WQ * 512
                    nc.sync.dma_start(out=dxh[:, lo:lo + WQ * 512], in_=ob)
                elif last_batch and v % 2 == 1:
                    # final batch: emit per pair to shrink the drain
                    lo = (v - 1) * 512
                    pr = ((v - 1) % WQ) // 2
                    nc.sync.dma_start(
                        out=dxh[:, lo:lo + 1024],
                        in_=ob[:, pr * 1024:(pr + 1) * 1024])
    nc.compile()
    return nc


def _hypernet(t, W1, b1, W2, b2, W3, b3):
    p = np.tanh(t.reshape(1, 1) @ W1 + b1)
    p = np.tanh(p @ W2 + b2)
    p = (p @ W3 + b3).reshape(-1).astype(np.float32)
    W = p[:BLOCK].reshape(E, D)
    U = p[BLOCK:2 * BLOCK].reshape(E, D)
    G = 1.0 / (1.0 + np.exp(-p[2 * BLOCK:3 * BLOCK].reshape(E, D)))
    U = (U * G).astype(np.float32)
    B = p[3 * BLOCK:].reshape(E, 1).astype(np.float32)
    return W.astype(np.float32), U, B


def kernel(t, x, W1, b1, W2, b2, W3, b3):
    W, U, B = _hypernet(
        np.asarray(t, np.float32), np.asarray(W1, np.float32),
        np.asarray(b1, np.float32), np.asarray(W2, np.float32),
        np.asarray(b2, np.float32), np.asarray(W3, np.float32),
        np.asarray(b3, np.float32),
    )
    wu = np.sum(W * U, axis=1).astype(np.float32)      # [E]

    cst = np.zeros((128, 258), np.float32)
    cst[0:64, 0:64] = W.T
    cst[64:128, 64:128] = W.T
    cst[0:64, 128:192] = U / E
    cst[64:128, 192:256] = U / E
    cst[0:64, 256] = wu
    cst[64:128, 257] = wu
    cst = cst.astype(np.float16)
    bdup = np.concatenate([B, B], axis=0).reshape(128, 1).astype(np.float32)
    cst = np.concatenate([cst, bdup.view(np.float16).reshape(128, 2)], axis=1)

    # x [N, D] -> per-core [128, NSH//2] fp16; sample (c, w, s, j) at
    # partition s*64+d, column w*512+j
    xs = np.asarray(x, np.float16).reshape(NCORES, NWIN, 2, 512, D)
    xs = np.ascontiguousarray(xs.transpose(0, 2, 4, 1, 3))
    xl = xs.reshape(NCORES, 128, NSH // 2)

    if "nc" not in _CACHED:
        _CACHED["nc"] = _build_nc()
    nc = _CACHED["nc"]

    in_maps = [
        {"xt": xl[c], "cst": cst}
        for c in range(NCORES)
    ]
    res = run_bass_kernel_spmd(nc, in_maps, core_ids=list(range(NCORES)))

    out = np.empty((N, D + 1), np.float32)
    od = out[:, :D].reshape(NCORES, NWIN, 2, 512, D)
    ol = out[:, D].reshape(NCORES, NWIN, 2, 512)
    sw = float(np.sum(wu))
    for c in range(NCORES):
        dxc = res.results[c]["dxh"].astype(np.float32)
        od[c] = dxc.reshape(2, D, NWIN, 512).transpose(2, 0, 3, 1)
        dlc = res.results[c]["dlh"]          # [66, 512*NDLG]
        # window w = g*DLG + k lives at rows 32k:32k+2, group col block g
        dlg = dlc.reshape(33, 2, NDLG, 512)[::16]      # [k, s, g, j]
        dlw = dlg.transpose(2, 0, 1, 3).reshape(NDLG * DLG, 2, 512)[:NWIN]
        ol[c] = (dlw - sw) / E
    return out
